# revision 10
# baseline (speedup 1.0000x reference)
"""Trainium2 Bass kernel for the DisentangledGNN problem (2-layer, 3-channel GNN).

Strategy (graph/data parallel over 8 NeuronCores):
  - Nodes sharded by contiguous range: core r owns targets [r*6250, (r+1)*6250).
    Shards padded to 6272 rows (49 full 128-tiles) -> padded node space (50176)
    used by all on-device tables.
  - Edges sharded by target core, laid out group-major ([g0-lo | g0-hi | g1-lo
    ...], where lo/hi split the source table at the int16-addressable row 32768),
    padded so every (group, half) run is a whole number of 128-edge blocks and
    block counts are uniform across cores (one SPMD program).
  - Scatter-adds become per-128-target-group segment sums computed as
    S^T @ msg matmuls accumulated in PSUM; S is built on device (iota == tgt).
  - Every core redundantly computes the node-level GEMMs
    H_cat = x @ [W_sp|W_ctx|W_lat] (bf16), scales rows by the per-channel
    source-side degree term, and stores the channel-interleaved table in its own
    HBM so per-edge source gathers (dma_gather, 768B rows) are local.
  - deg/dinv: per-channel degrees segment-summed locally, AllGathered;
    dinv = exp(p * ln(deg)) computed on device per layer/channel.
  - Between layers: LN + ReLU on the local out shard, PE-transpose, AllGather
    (bf16, 4 chunks overlapping the edge phase) -> next layer's GEMM input.
"""

import sys

sys.path.insert(0, "/opt/trn_rl_repo")

from contextlib import ExitStack

import numpy as np
import ml_dtypes

import os as _os
DBG_NO_GATHER = _os.environ.get("K_NO_GATHER") == "1"
DBG_NO_COLL = _os.environ.get("K_NO_COLL") == "1"
DBG_NO_EDGEMM = _os.environ.get("K_NO_EDGEMM") == "1"
import concourse.bacc as bacc
import concourse.bass as bass
import concourse.mybir as mybir
import concourse.tile as tile
from concourse.bass_utils import run_bass_kernel_spmd

F32 = mybir.dt.float32
BF16 = mybir.dt.bfloat16
I16 = mybir.dt.int16
AX = mybir.AxisListType
ALU = mybir.AluOpType
ACTF = mybir.ActivationFunctionType

NC = 8          # cores
D = 128         # feature dim
CH = 3          # edge channels
FD = CH * D     # 384: channel-interleaved feature width
WIN = 16        # gather window size in 128-edge blocks
LN_EPS = 1e-5
SPLIT = 32768   # int16-addressable row limit for dma_gather tables


def _ceil_div(a, b):
    return (a + b - 1) // b


def _pack_idx16(flat_idx):
    """Flat idx stream (len 128*nblocks) -> [128, 8*nblocks] int16 in the
    dma_gather wrapped layout (idx i -> partition i%16, col i//16, replicated
    over the 8 groups of 16 partitions)."""
    n = flat_idx.shape[0]
    a = flat_idx.reshape(n // 16, 16).T.astype(np.int16)
    return np.tile(a, (8, 1))


def _shard_major(a, p=128):
    """[nblk*128, k] -> [128, nblk*k] with col b*k+c, partition = pos%128."""
    nb = a.shape[0] // p
    return np.ascontiguousarray(a.reshape(nb, p, -1).transpose(1, 0, 2).reshape(p, -1))


class _Plan:
    """Host-side static structure (uniform across cores)."""

    def __init__(self, N, E, src, tgt):
        self.N, self.E = N, E
        SH = N // NC
        G = _ceil_div(SH, 128)
        SHP = G * 128
        NP = NC * SHP
        self.SH, self.G, self.SHP, self.NP = SH, G, SHP, NP

        r = tgt // SH
        tl = tgt - r * SH
        g = tl >> 7
        t128 = tl & 127
        spad = (src // SH) * SHP + (src % SH)
        self.ESPLIT = min(SPLIT, NP)
        half = (spad >= self.ESPLIT).astype(np.int64)

        seg = (r * G + g) * 2 + half
        order = np.argsort(seg * np.int64(NP) + spad, kind="stable")
        self.order = order
        seg_s = seg[order]

        nseg = NC * G * 2
        cnt = np.bincount(seg_s, minlength=nseg).reshape(NC, G, 2)
        mx = cnt.max(axis=0)                       # [G, 2]
        nbl = _ceil_div(mx[:, 0], 128).astype(np.int64)
        nbh = _ceil_div(mx[:, 1], 128).astype(np.int64)
        for gg in range(G):                        # insurance: >=1 block/group
            if nbl[gg] + nbh[gg] == 0:
                nbl[gg] = 1
        self.NBL, self.NBH = nbl, nbh

        # group-major combined stream: [g-lo blocks | g-hi blocks] per group
        nbg = nbl + nbh
        gb0 = np.concatenate([[0], np.cumsum(nbg)[:-1]])   # first block of group
        self.gb0 = gb0
        self.TOTB = int(nbg.sum())

        # per-block half flag
        half_of_blk = np.zeros(self.TOTB, np.int64)
        for gg in range(G):
            half_of_blk[gb0[gg] + nbl[gg]: gb0[gg] + nbg[gg]] = 1
        self.half_of_blk = half_of_blk

        # flat slot of each sorted edge within its core's stream
        seg_starts = np.concatenate([[0], np.cumsum(cnt.reshape(-1))[:-1]])
        pos_in_seg = np.arange(E) - seg_starts[seg_s]
        g_s = (seg_s // 2) % G
        h_s = seg_s % 2
        base_blk = gb0[g_s] + np.where(h_s == 1, nbl[g_s], 0)
        self.slot_s = base_blk * 128 + pos_in_seg
        self.core_s = seg_s // (2 * G)
        spad_s = spad[order]
        self.idxval_s = np.where(h_s == 0, spad_s, spad_s - self.ESPLIT)
        self.t128_s = t128[order]

        # group -> block range
        self.group_blocks = [list(range(int(gb0[gg]), int(gb0[gg] + nbg[gg])))
                             for gg in range(G)]
        # windows: 16-block tiles of the stream; segments = same-half runs
        self.windows = []
        for b0 in range(0, self.TOTB, WIN):
            nb = min(WIN, self.TOTB - b0)
            segs = []
            j = 0
            while j < nb:
                h = half_of_blk[b0 + j]
                j1 = j
                while j1 < nb and half_of_blk[b0 + j1] == h:
                    j1 += 1
                segs.append((j, j1 - j, int(h)))
                j = j1
            self.windows.append((b0, nb, segs))


def _build_core_inputs(plan, inputs):
    E = plan.E
    TOT = plan.TOTB * 128
    alphas = np.stack([inputs["alpha_sp"], inputs["alpha_ctx"], inputs["alpha_lat"]], 1)
    ws = np.stack([inputs["w_sp"], inputs["w_ctx"], inputs["w_lat"]], 1)
    alphas_s = np.asarray(alphas, np.float32)[plan.order]
    ws_s = np.asarray(ws, np.float32)[plan.order]

    per_core = []
    for r in range(NC):
        m = plan.core_s == r
        slot = plan.slot_s[m]
        fi = np.zeros(TOT, np.int64)
        fa = np.zeros((TOT, CH), np.float32)
        fw = np.zeros((TOT, CH), np.float32)
        ft = np.zeros(TOT, np.float32)
        fi[slot] = plan.idxval_s[m]
        fa[slot] = alphas_s[m]
        fw[slot] = ws_s[m]
        ft[slot] = plan.t128_s[m]
        per_core.append({
            "idx16": _pack_idx16(fi),
            "a3": _shard_major(fa),
            "w3": _shard_major(fw),
            "tl": _shard_major(ft[:, None]).astype(ml_dtypes.bfloat16),
        })
    return per_core


def _build_nc(plan, L):
    G, SHP, NP, TOTB = plan.G, plan.SHP, plan.NP, plan.TOTB

    nc = bacc.Bacc("TRN2", target_bir_lowering=False, debug=False, num_devices=NC)

    xT_in = nc.dram_tensor("xT", [D, NP], BF16, kind="ExternalInput")
    xTloc_in = nc.dram_tensor("xTloc", [D, SHP], BF16, kind="ExternalInput")
    wcat_in = nc.dram_tensor("wcat", [D, L * FD], BF16, kind="ExternalInput")
    wself_in = nc.dram_tensor("wself", [D, L * D], BF16, kind="ExternalInput")
    bias_in = nc.dram_tensor("biasbc", [D, L * D], F32, kind="ExternalInput")
    gbc_in = nc.dram_tensor("gbc", [D, D], F32, kind="ExternalInput")
    bbc_in = nc.dram_tensor("bbc", [D, D], F32, kind="ExternalInput")
    dp_in = nc.dram_tensor("dp", [D, L * CH], F32, kind="ExternalInput")
    iota_in = nc.dram_tensor("iotabf", [D, D], BF16, kind="ExternalInput")
    ident_in = nc.dram_tensor("identf", [D, D], F32, kind="ExternalInput")
    eps_in = nc.dram_tensor("epscol", [D, 1], F32, kind="ExternalInput")
    idx_in = nc.dram_tensor("idx16", [D, 8 * TOTB], I16, kind="ExternalInput")
    a3_in = nc.dram_tensor("a3", [D, CH * TOTB], F32, kind="ExternalInput")
    w3_in = nc.dram_tensor("w3", [D, CH * TOTB], F32, kind="ExternalInput")
    tl_in = nc.dram_tensor("tl", [D, TOTB], BF16, kind="ExternalInput")
    out_dram = nc.dram_tensor("out", [SHP, D], F32, kind="ExternalOutput")

    # AllGather chunks of h1T by target group
    AGCH = []
    gg0 = 0
    while gg0 < G:
        ng = min(13, G - gg0)
        AGCH.append((gg0, ng))
        gg0 += ng
    grp_chunk = {}
    for k, (c0, ng) in enumerate(AGCH):
        for gg in range(c0, c0 + ng):
            grp_chunk[gg] = k

    with tile.TileContext(nc) as tc, ExitStack() as stack:
        dram = stack.enter_context(tc.tile_pool(name="dram", bufs=1, space="DRAM"))
        hcat = [dram.tile([NP, FD], BF16, name=f"hcat{l}", tag=f"hcat{l}")
                for l in range(L)]
        deg_bn = dram.tile([SHP, CH], F32, name="deg_bn")
        deg_all = dram.tile([NC, SHP, CH], F32, name="deg_all", addr_space="Shared")
        ag_bn = [dram.tile([D, ng * D], BF16, name=f"agbn{k}")
                 for k, (c0, ng) in enumerate(AGCH)]
        h1ag = [dram.tile([NC, D, ng * D], BF16, name=f"h1ag{k}", addr_space="Shared")
                for k, (c0, ng) in enumerate(AGCH)]

        const = stack.enter_context(tc.tile_pool(name="const", bufs=1))
        iota = const.tile([D, D], BF16, name="iota")
        ident = const.tile([D, D], F32, name="ident")
        epsc = const.tile([D, 1], F32, name="epsc")
        dp = const.tile([D, L * CH], F32, name="dp_sb")
        gbc = const.tile([D, D], F32, name="gbc_sb")
        bbc = const.tile([D, D], F32, name="bbc_sb")
        biasbc = const.tile([D, L * D], F32, name="biasbc_sb")
        wcat = const.tile([D, L * FD], BF16, name="wcat_sb")
        wself = const.tile([D, L * D], BF16, name="wself_sb")
        idx_sb = const.tile([D, 8 * TOTB], I16, name="idx_sb")
        tl_sb = const.tile([D, TOTB], BF16, name="tl_sb")
        eff3b = const.tile([D, CH * TOTB], BF16, name="eff3b")
        xTloc = const.tile([D, SHP], BF16, name="xTloc_sb")
        h1Tloc = const.tile([D, SHP], BF16, name="h1Tloc")
        self_sb = const.tile([D, G * D], F32, name="self_sb")
        deg_sb = const.tile([D, G * CH], F32, name="deg_sb")
        NG = NC * G
        degT = const.tile([D, NG * CH], F32, name="degT")
        lnd = const.tile([D, NG * CH], F32, name="lnd")
        lnloc = const.tile([D, G * CH], F32, name="lnloc")
        dinvG = [const.tile([D, CH * NG], F32, name=f"dinvG{l}") for l in range(L)]
        dinvL = [const.tile([D, CH * G], F32, name=f"dinvL{l}") for l in range(L)]

        for dst, srct in ((iota, iota_in), (ident, ident_in), (epsc, eps_in), (dp, dp_in),
                          (gbc, gbc_in), (bbc, bbc_in), (biasbc, bias_in),
                          (wcat, wcat_in), (wself, wself_in), (idx_sb, idx_in),
                          (tl_sb, tl_in), (xTloc, xTloc_in)):
            nc.sync.dma_start(dst[:], srct[:])

        # ---- eff = alpha * w ----
        with tc.tile_pool(name="effload", bufs=1) as effp:
            a3t = effp.tile([D, CH * TOTB], F32, name="a3t")
            w3t = effp.tile([D, CH * TOTB], F32, name="w3t")
            nc.sync.dma_start(a3t[:], a3_in[:])
            nc.sync.dma_start(w3t[:], w3_in[:])
            hc = (CH * TOTB + 1) // 2
            for c0 in (0, hc):
                c1 = min(c0 + hc, CH * TOTB)
                nc.vector.tensor_tensor(eff3b[:, c0:c1], a3t[:, c0:c1],
                                        w3t[:, c0:c1], op=ALU.mult)

        def bcast_ap(t, dims, off=0):
            return bass.AP(tensor=t.tensor, offset=t.offset + off,
                           ap=[t.ap[0]] + dims)

        def build_S(Spool, wi):
            b0, nb, _segs = plan.windows[wi]
            S_w = Spool.tile([D, WIN, D], BF16, name=f"S{wi}", tag="S")
            nc.vector.tensor_tensor(
                S_w[:, :nb, :],
                bcast_ap(iota, [[0, nb], [1, D]]),
                bcast_ap(tl_sb, [[1, nb], [0, D]], off=b0),
                op=ALU.is_equal)
            return S_w

        # ================= deg pass =================
        with tc.tile_pool(name="degS", bufs=3) as Spool, \
             tc.tile_pool(name="degpsum", bufs=2, space="PSUM") as dpp:
            cur = {}

            def deg_S(wi):
                if wi not in cur:
                    cur.clear()
                    cur[wi] = build_S(Spool, wi)
                return cur[wi]

            for gg in range(G):
                blks = plan.group_blocks[gg]
                pt = dpp.tile([D, 16], F32, name=f"dps{gg}", tag="dps")
                for i, b in enumerate(blks):
                    S_w = deg_S(b // WIN)
                    nc.tensor.matmul(pt[:, 0:CH], S_w[:, b % WIN, :],
                                     eff3b[:, CH * b:CH * (b + 1)],
                                     start=(i == 0), stop=(i == len(blks) - 1))
                nc.scalar.activation(deg_sb[:, CH * gg:CH * (gg + 1)], pt[:, 0:CH],
                                     ACTF.Identity, bias=1.0)

        nc.sync.dma_start(
            deg_bn.rearrange("(g p) c -> p g c", p=D),
            deg_sb.rearrange("p (g c) -> p g c", c=CH))
        if DBG_NO_COLL:
            for _r in range(NC):
                nc.sync.dma_start(deg_all[_r], deg_bn[:])
        else:
            nc.gpsimd.collective_compute(
                "AllGather", ALU.bypass, replica_groups=[list(range(NC))],
                ins=[deg_bn.opt()], outs=[deg_all.opt()])
        nc.sync.dma_start(
            degT.rearrange("p (r g c) -> p r g c", r=NC, c=CH),
            deg_all.rearrange("r (g p) c -> p r g c", p=D))

        nc.vector.tensor_scalar(lnd[:], degT[:], 1e-6, None, op0=ALU.max)
        nc.scalar.activation(lnd[:], lnd[:], ACTF.Ln)
        nc.vector.tensor_scalar(lnloc[:], deg_sb[:], 1e-6, None, op0=ALU.max)
        nc.scalar.activation(lnloc[:], lnloc[:], ACTF.Ln)
        for l in range(L):
            for c in range(CH):
                sc = dp[:, l * CH + c:l * CH + c + 1]
                nc.scalar.activation(
                    dinvG[l][:, c * NG:(c + 1) * NG],
                    lnd.rearrange("p (t c) -> p c t", c=CH)[:, c, :],
                    ACTF.Exp, scale=sc)
                nc.scalar.activation(
                    dinvL[l][:, c * G:(c + 1) * G],
                    lnloc.rearrange("p (g c) -> p c g", c=CH)[:, c, :],
                    ACTF.Exp, scale=sc)

        # ================= layers =================
        for l in range(L):
            # ---- H_cat GEMM over all padded nodes ----
            with tc.tile_pool(name=f"gl{l}", bufs=2) as glp, \
                 tc.tile_pool(name=f"gst{l}", bufs=2) as gst, \
                 tc.tile_pool(name=f"gps{l}", bufs=2, space="PSUM") as gps:
                state = {"t": 0, "stage": None, "n": 0, "t0": 0}

                def flush_stage(state=state, l=l):
                    if state["stage"] is not None and state["n"] > 0:
                        nc.sync.dma_start(
                            hcat[l][state["t0"] * D:(state["t0"] + state["n"]) * D, :]
                            .rearrange("(b p) e -> p b e", p=D),
                            state["stage"][:, :state["n"], :])
                    state["stage"] = None
                    state["n"] = 0

                def do_tile(lhsT_ap, state=state, l=l):
                    t = state["t"]
                    pg = gps.tile([D, FD], F32, name=f"pg{l}_{t}", tag="pg")
                    nc.tensor.matmul(pg[:], lhsT_ap, wcat[:, l * FD:(l + 1) * FD],
                                     start=True, stop=True)
                    if state["stage"] is None:
                        state["stage"] = gst.tile([D, 8, FD], BF16,
                                                  name=f"hst{l}_{t}", tag="hst")
                        state["t0"] = t
                        state["n"] = 0
                    st = state["stage"]
                    for c in range(CH):
                        sc = dinvG[l][:, c * NG + t:c * NG + t + 1]
                        dst = st[:, state["n"], c * D:(c + 1) * D]
                        srcp = pg[:, c * D:(c + 1) * D]
                        if c == 1:
                            nc.scalar.activation(dst, srcp, ACTF.Copy, scale=sc)
                        else:
                            nc.vector.tensor_scalar_mul(dst, srcp, sc)
                    state["n"] += 1
                    state["t"] += 1
                    if state["n"] == 8:
                        flush_stage()

                if l == 0:
                    CHK = NP // 8
                    for c0 in range(0, NP, CHK):
                        xc = glp.tile([D, CHK], BF16, name=f"xc{c0}", tag="xc")
                        nc.sync.dma_start(xc[:], xT_in[:, c0:c0 + CHK])
                        for t in range(CHK // D):
                            do_tile(xc[:, t * D:(t + 1) * D])
                else:
                    for rr in range(NC):
                        for k, (c0, ng) in enumerate(AGCH):
                            xc = glp.tile([D, 13 * D], BF16,
                                          name=f"hc{rr}_{k}", tag="xc")
                            nc.sync.dma_start(xc[:, :ng * D], h1ag[k][rr, :, :])
                            for t in range(ng):
                                do_tile(xc[:, t * D:(t + 1) * D])
                flush_stage()

                # ---- self GEMM (local shard) ----
                src_loc = xTloc if l == 0 else h1Tloc
                for gg in range(G):
                    ps = gps.tile([D, D], F32, name=f"ps{l}_{gg}", tag="ps")
                    nc.tensor.matmul(ps[:], src_loc[:, gg * D:(gg + 1) * D],
                                     wself[:, l * D:(l + 1) * D],
                                     start=True, stop=True)
                    nc.vector.tensor_tensor(self_sb[:, gg * D:(gg + 1) * D],
                                            ps[:], biasbc[:, l * D:(l + 1) * D],
                                            op=ALU.add)

            # ---- edge pass ----
            with tc.tile_pool(name=f"ed{l}", bufs=2) as edp, \
                 tc.tile_pool(name=f"eS{l}", bufs=2) as eSp, \
                 tc.tile_pool(name=f"emsg{l}", bufs=2) as emp, \
                 tc.tile_pool(name=f"eout{l}", bufs=3) as eop, \
                 tc.tile_pool(name=f"eps{l}", bufs=2, space="PSUM") as epp, \
                 tc.tile_pool(name=f"ept{l}", bufs=2, space="PSUM") as ptp, \
                 tc.tile_pool(name=f"est{l}", bufs=2) as ostp:

                win_data = {}

                def ensure_window(wi, l=l, win_data=win_data):
                    if wi in win_data:
                        return win_data[wi]
                    b0, nb, segs = plan.windows[wi]
                    dst = edp.tile([D, WIN, FD], BF16, name=f"gd{l}_{wi}", tag="gd")
                    if not DBG_NO_GATHER:
                        # dma_gather wedges the device above 1024 indices per
                        # call -- split segments into <=8-block subcalls.
                        for (j0, nsb, h) in segs:
                            view = hcat[l][plan.ESPLIT:NP, :] if h else hcat[l][0:plan.ESPLIT, :]
                            for s0 in range(j0, j0 + nsb, 8):
                                sn = min(8, j0 + nsb - s0)
                                nidx = sn * 128
                                nc.gpsimd.dma_gather(
                                    dst[:, s0:s0 + sn, :], view,
                                    idx_sb[:, (b0 + s0) * 8:(b0 + s0 + sn) * 8],
                                    nidx, nidx, FD)
                    else:
                        nc.vector.memset(dst[:, 0:nb, :], 0.25)
                    msg = emp.tile([D, WIN, FD], BF16, name=f"mg{l}_{wi}", tag="mg")
                    m4 = msg.rearrange("p b (c f) -> p b c f", c=CH)
                    d4 = dst.rearrange("p b (c f) -> p b c f", c=CH)
                    nc.vector.tensor_tensor(
                        m4[:, :nb, :, :], d4[:, :nb, :, :],
                        bcast_ap(eff3b, [[CH, nb], [1, CH], [0, D]], off=CH * b0),
                        op=ALU.mult)
                    S_w = build_S(eSp, wi)
                    win_data.clear()
                    win_data[wi] = (S_w, msg)
                    return win_data[wi]

                ost = {"stage": None, "n": 0, "g0": 0}

                def flush_out(ost=ost):
                    if ost["stage"] is not None and ost["n"] > 0:
                        nc.sync.dma_start(
                            out_dram[ost["g0"] * D:(ost["g0"] + ost["n"]) * D, :]
                            .rearrange("(b p) f -> p b f", p=D),
                            ost["stage"][:, :ost["n"], :])
                    ost["stage"] = None
                    ost["n"] = 0

                for gg in range(G):
                    blks = plan.group_blocks[gg]
                    pe = epp.tile([D, FD], F32, name=f"pe{l}_{gg}", tag="pe")
                    if DBG_NO_EDGEMM:
                        for i, b in enumerate(blks):
                            ensure_window(b // WIN)
                        nc.vector.memset(pe[:], 0.0)
                    else:
                        for i, b in enumerate(blks):
                            S_w, msg = ensure_window(b // WIN)
                            j = b % WIN
                            nc.tensor.matmul(pe[:], S_w[:, j, :], msg[:, j, :],
                                             start=(i == 0), stop=(i == len(blks) - 1))
                    t0 = eop.tile([D, D], F32, name=f"t0_{l}_{gg}", tag="t0")
                    t1 = eop.tile([D, D], F32, name=f"t1_{l}_{gg}", tag="t1")
                    t2 = eop.tile([D, D], F32, name=f"t2_{l}_{gg}", tag="t2")
                    ot = eop.tile([D, D], F32, name=f"ot_{l}_{gg}", tag="ot")
                    nc.vector.tensor_scalar_mul(t0[:], pe[:, 0:D],
                                                dinvL[l][:, gg:gg + 1])
                    nc.scalar.activation(t1[:], pe[:, D:2 * D], ACTF.Copy,
                                         scale=dinvL[l][:, G + gg:G + gg + 1])
                    nc.vector.tensor_scalar_mul(t2[:], pe[:, 2 * D:3 * D],
                                                dinvL[l][:, 2 * G + gg:2 * G + gg + 1])
                    nc.vector.tensor_tensor(t0[:], t0[:], t1[:], op=ALU.add)
                    nc.vector.tensor_tensor(t2[:], t2[:],
                                            self_sb[:, gg * D:(gg + 1) * D], op=ALU.add)
                    nc.vector.tensor_tensor(ot[:], t0[:], t2[:], op=ALU.add)

                    if l < L - 1:
                        stats = eop.tile([D, 8], F32, name=f"st_{l}_{gg}", tag="st")
                        sqs = eop.tile([D, D], F32, name=f"sq_{l}_{gg}", tag="sq")
                        nc.vector.reduce_sum(stats[:, 0:1], ot[:], axis=AX.X)
                        nc.scalar.activation(sqs[:], ot[:], ACTF.Square,
                                             accum_out=stats[:, 1:2])
                        nc.vector.tensor_scalar_mul(stats[:, 2:3], stats[:, 0:1], 1.0 / D)
                        nc.vector.tensor_scalar_mul(stats[:, 3:4], stats[:, 1:2], 1.0 / D)
                        nc.vector.tensor_tensor(stats[:, 4:5], stats[:, 2:3],
                                                stats[:, 2:3], op=ALU.mult)
                        nc.vector.tensor_tensor(stats[:, 5:6], stats[:, 3:4],
                                                stats[:, 4:5], op=ALU.subtract)
                        nc.scalar.activation(stats[:, 6:7], stats[:, 5:6],
                                             ACTF.Sqrt, bias=epsc[:, 0:1])
                        nc.vector.reciprocal(stats[:, 7:8], stats[:, 6:7])
                        hn = eop.tile([D, D], F32, name=f"hn_{l}_{gg}", tag="hn")
                        nc.vector.tensor_scalar(hn[:], ot[:], stats[:, 2:3],
                                                stats[:, 7:8],
                                                op0=ALU.subtract, op1=ALU.mult)
                        nc.vector.tensor_tensor(hn[:], hn[:], gbc[:], op=ALU.mult)
                        nc.vector.tensor_tensor(hn[:], hn[:], bbc[:], op=ALU.add)
                        nc.scalar.activation(hn[:], hn[:], ACTF.Relu)
                        ptt = ptp.tile([D, D], F32, name=f"pt_{l}_{gg}", tag="pt")
                        nc.tensor.transpose(ptt[:], hn[:], ident[:])
                        nc.scalar.copy(h1Tloc[:, gg * D:(gg + 1) * D], ptt[:])
                        k = grp_chunk[gg]
                        c0, ngk = AGCH[k]
                        if gg == c0 + ngk - 1:
                            nc.sync.dma_start(ag_bn[k][:],
                                              h1Tloc[:, c0 * D:(c0 + ngk) * D])
                            if DBG_NO_COLL:
                                for _r in range(NC):
                                    nc.sync.dma_start(h1ag[k][_r], ag_bn[k][:])
                            else:
                                nc.gpsimd.collective_compute(
                                    "AllGather", ALU.bypass,
                                    replica_groups=[list(range(NC))],
                                    ins=[ag_bn[k].opt()], outs=[h1ag[k].opt()])
                    else:
                        if ost["stage"] is None:
                            ost["stage"] = ostp.tile([D, 8, D], F32,
                                                     name=f"ost{gg}", tag="ost")
                            ost["g0"] = gg
                            ost["n"] = 0
                        nc.vector.tensor_copy(ost["stage"][:, ost["n"], :], ot[:])
                        ost["n"] += 1
                        if ost["n"] == 8:
                            flush_out()
                if l == L - 1:
                    flush_out()

    nc.compile()
    return nc


def _prep_shared_inputs(plan, inputs, L):
    SH, SHP, NP = plan.SH, plan.SHP, plan.NP
    x = np.asarray(inputs["x"], np.float32)
    xp = np.zeros((NP, D), np.float32)
    for r in range(NC):
        xp[r * SHP:r * SHP + SH] = x[r * SH:(r + 1) * SH]
    xT = np.ascontiguousarray(xp.T).astype(ml_dtypes.bfloat16)
    W3 = np.asarray(inputs["W3"], np.float32)
    Wself = np.asarray(inputs["W_self"], np.float32)
    bias = np.asarray(inputs["bias"], np.float32)
    dpow = np.asarray(inputs["deg_power"], np.float32)
    ln_g = np.asarray(inputs["ln_g"], np.float32)
    ln_b = np.asarray(inputs["ln_b"], np.float32)

    # [D, L*FD]: cols l*FD + c*D + f  = W3[l, c, :, f]
    wcat = np.concatenate(
        [np.concatenate([W3[l, c] for c in range(CH)], axis=1) for l in range(L)],
        axis=1)
    wself_cols = np.concatenate([Wself[l] for l in range(L)], axis=1)
    biasbc = np.concatenate(
        [np.tile(bias[l][None, :], (D, 1)) for l in range(L)], axis=1)
    shared = {
        "xT": xT,
        "wcat": wcat.astype(ml_dtypes.bfloat16),
        "wself": wself_cols.astype(ml_dtypes.bfloat16),
        "biasbc": biasbc.astype(np.float32),
        "gbc": np.tile(ln_g[None, :], (D, 1)).astype(np.float32),
        "bbc": np.tile(ln_b[None, :], (D, 1)).astype(np.float32),
        "dp": np.tile(dpow.reshape(1, L * CH), (D, 1)).astype(np.float32),
        "iotabf": np.tile(np.arange(D), (D, 1)).astype(ml_dtypes.bfloat16),
        "identf": np.eye(D, dtype=np.float32),
        "epscol": np.full((D, 1), LN_EPS, np.float32),
    }
    return shared, xT


def run(inputs, trace=False, tmpdir=None):
    N = int(inputs["num_nodes"])
    edge_index = np.asarray(inputs["edge_index"])
    src = edge_index[0].astype(np.int64)
    tgt = edge_index[1].astype(np.int64)
    E = src.shape[0]
    L = int(np.asarray(inputs["W3"]).shape[0])
    assert N % NC == 0

    plan = _Plan(N, E, src, tgt)
    nc = _build_nc(plan, L)
    shared, xT = _prep_shared_inputs(plan, inputs, L)
    per_core = _build_core_inputs(plan, inputs)

    in_maps = []
    for r in range(NC):
        m = dict(shared)
        m["xTloc"] = np.ascontiguousarray(xT[:, r * plan.SHP:(r + 1) * plan.SHP])
        m.update(per_core[r])
        in_maps.append(m)

    kw = {}
    if trace:
        kw.update(trace=True, tmpdir=tmpdir)
    res = run_bass_kernel_spmd(nc, in_maps, core_ids=list(range(NC)), **kw)
    out = np.concatenate([res.results[r]["out"][:plan.SH] for r in range(NC)], axis=0)
    return np.ascontiguousarray(out.astype(np.float32)), res


def kernel(**inputs) -> np.ndarray:
    out, _ = run(inputs)
    return out
